# revision 22
# baseline (speedup 1.0000x reference)
"""AdaptiveSpectralFeatureRefinement (Euclidean) — Trainium2 Bass kernel.

Reference op (per batch element b):
  patches = unfold3x3(fused_features)                 # [C, 9, H, W]
  dist_k  = || patches_k - fe_lv ||_2  (over C)       # [9, H, W]
  w       = softmax_k(-dist_k)
  out     = sum_k w_k * patches_k + fe_lv             # [C, H, W]

Sharding: data-parallel over batch B=8 across the 8 NeuronCores (the op is
fully local per batch element, no collectives needed).

Per-core layout: partitions = w (128 cols), free = (h, c) with c innermost.
  - DRAM loads are *natural* (partition = c, 64KB contiguous runs) and then
    transposed on-chip through the TensorEngine ([64,128] blocks -> [128,64])
    because a direct transposing DMA load needs 8192 x 512B descriptors which
    overflows the compiler's 16-bit DMA semaphore field.
  - dx (col) shifts -> three partition-shifted copies of f (SBUF->SBUF DMA)
  - dy (row) shifts -> free-dim offset slices into an h-padded [*, H+2, C] tile
"""

import sys

if "/opt/trn_rl_repo" not in sys.path:
    sys.path.insert(0, "/opt/trn_rl_repo")

import os
from contextlib import ExitStack

import numpy as np

import concourse.bass as bass
import concourse.tile as tile
from concourse import mybir
from concourse.bass_utils import run_bass_kernel_spmd

B, C, H, W = 8, 64, 128, 128
N_CORES = 8
FP = mybir.dt.float32
ACT = mybir.ActivationFunctionType

_cache = {}


def _split_sync_waits(nc, max_waits=1):
    """This container's walrus codegen accepts at most one sync-wait command
    per instruction, but Tile emits up to ~3 on instructions with multiple
    cross-engine producers.  Legalize by hoisting the extra waits into NoOps
    on the same engine, inserted immediately before the instruction (engine
    streams execute in block order, so the waits still gate it)."""
    for f in nc.m.functions:
        for blk in f.blocks:
            new_insts = []
            changed = False
            for inst in blk.instructions:
                si = getattr(inst, "sync_info", None)
                if si is not None and si.on_wait and len(si.on_wait) > max_waits:
                    waits = list(si.on_wait)
                    for i, w in enumerate(waits[max_waits:]):
                        nop = mybir.InstNoOp(
                            name=f"{inst.name}_ws{i}",
                            engine=inst.engine,
                            sync_info=mybir.SyncInfo(on_wait=[w],
                                                     on_update=[]),
                            bass_nofuse=True,
                        )
                        new_insts.append(nop)
                    inst.sync_info = mybir.SyncInfo(
                        on_wait=waits[:max_waits],
                        on_update=list(si.on_update),
                    )
                    changed = True
                new_insts.append(inst)
            if changed:
                blk.instructions = new_insts
    return nc


def _build_kernel():
    nc = bass.Bass("TRN2", target_bir_lowering=False, debug=False,
                   num_devices=N_CORES)

    x_d = nc.dram_tensor("fe_lv", [C, H, W], FP, kind="ExternalInput").ap()
    f_d = nc.dram_tensor("fused_features", [C, H, W], FP,
                         kind="ExternalInput").ap()
    o_d = nc.dram_tensor("out", [C, H, W], FP, kind="ExternalOutput").ap()

    # DRAM APs reordered to [h, c, w] so partition dim = h.
    x_hcw = x_d.transpose([1, 0, 2])
    f_hcw = f_d.transpose([1, 0, 2])
    o_hcw = o_d.transpose([1, 0, 2])

    with tile.TileContext(nc) as tc, ExitStack() as ctx:
        main = ctx.enter_context(tc.tile_pool(name="main", bufs=1))

        # Persistent tiles; layout [h(part), c, w(+2 pad)].
        x_sb = main.tile([128, C, W], FP)
        f_m1 = main.tile([128, C, W + 2], FP)   # f rows h-1 (w-padded)
        f_c0 = main.tile([128, C, W + 2], FP)   # f rows h
        f_p1 = main.tile([128, C, W + 2], FP)   # f rows h+1
        diff = main.tile([128, C, W], FP)       # scratch / wsum tmp
        acc = main.tile([128, C, W], FP)        # wsum accumulator
        dist = main.tile([128, 9, W], FP)       # dist^2 -> dist
        ew = main.tile([128, 9, W], FP)         # exp weights
        mmin = main.tile([128, W], FP)
        ssum = main.tile([128, W], FP)

        f_dy = {-1: f_m1, 0: f_c0, 1: f_p1}

        # ---- loads ----
        # A DMA instruction supports only one sync-wait, so each f copy is an
        # independent DRAM load whose only producer is its own tile's memset
        # (which provides the zero w pads and boundary rows).
        nc.sync.dma_start(out=x_sb[:, :, :], in_=x_hcw)
        # Negate x in place on DVE right after the load: the first phase-1 op
        # then sees x as a DVE-produced tile (same-engine dep, no semaphore),
        # keeping every TensorTensor within its 2 sync-wait slots.  Phase 1
        # computes f_k + (-x); the final residual uses acc - (-x).
        nc.vector.tensor_scalar_mul(x_sb[:, :, :], x_sb[:, :, :], -1.0)
        # memsets on DVE: the first consumer (tensor_sub) also runs on DVE,
        # so this producer needs no cross-engine semaphore wait (the TT
        # instruction has only 2 sync-wait slots, used by the two DMA loads).
        nc.vector.memset(f_c0[:, :, :], 0.0)
        nc.vector.memset(f_m1[:, :, :], 0.0)
        nc.vector.memset(f_p1[:, :, :], 0.0)
        nc.sync.dma_start(out=f_c0[:, :, 1:W + 1], in_=f_hcw)
        nc.sync.dma_start(out=f_m1[1:128, :, 1:W + 1],
                          in_=f_hcw[0:127, :, :])
        nc.sync.dma_start(out=f_p1[0:127, :, 1:W + 1],
                          in_=f_hcw[1:128, :, :])

        # Absorb each f-load's DMA completion into a DVE touch op: hardware
        # instructions have only 2 sync-wait slots, and these make every
        # downstream compute op's producers DVE-only (same-engine => no
        # semaphore).  (x's load is absorbed by the negate above.)
        for t in (f_m1, f_c0, f_p1):
            # TT-bypass identity: TensorTensor has 2 sync-wait slots while
            # TensorScalar has only 1.
            nc.vector.tensor_tensor(t[:, :, :], t[:, :, :], t[:, :, :],
                                    op=mybir.AluOpType.bypass)

        # ---- phase 1: dist^2 for the 9 neighbors ----
        for k in range(9):
            dy, dx = k // 3 - 1, k % 3 - 1
            f_k = f_dy[dy][:, :, 1 + dx:1 + dx + W]
            nc.vector.tensor_add(diff[:, :, :], f_k, x_sb[:, :, :])
            nc.scalar.activation(diff[:, :, :], diff[:, :, :], ACT.Square)
            # per-(h,w) sum over c: view [p, w, c], reduce innermost.
            nc.vector.tensor_reduce(
                out=dist[:, k, :], in_=diff[:, :, :].transpose([0, 2, 1]),
                axis=mybir.AxisListType.X, op=mybir.AluOpType.add,
            )

        # ---- phase 2: softmax over 9 neighbors of -sqrt(dist2) ----
        nc.vector.tensor_reduce(
            out=mmin[:, :], in_=dist[:, :, :].transpose([0, 2, 1]),
            axis=mybir.AxisListType.X, op=mybir.AluOpType.min,
        )
        nc.scalar.activation(dist[:, :, :], dist[:, :, :], ACT.Sqrt)
        nc.scalar.activation(mmin[:, :], mmin[:, :], ACT.Sqrt)
        # e = exp(dmin - d) (<= 1, no overflow)
        nc.vector.tensor_sub(
            ew[:, :, :],
            mmin[:, :].unsqueeze(1).broadcast_to([128, 9, W]),
            dist[:, :, :],
        )
        nc.scalar.activation(ew[:, :, :], ew[:, :, :], ACT.Exp)
        nc.vector.tensor_reduce(
            out=ssum[:, :], in_=ew[:, :, :].transpose([0, 2, 1]),
            axis=mybir.AxisListType.X, op=mybir.AluOpType.add,
        )
        nc.vector.reciprocal(ssum[:, :], ssum[:, :])
        nc.vector.tensor_mul(
            ew[:, :, :], ew[:, :, :],
            ssum[:, :].unsqueeze(1).broadcast_to([128, 9, W]),
        )

        # ---- phase 3: weighted sum + residual ----
        for k in range(9):
            dy, dx = k // 3 - 1, k % 3 - 1
            f_k = f_dy[dy][:, :, 1 + dx:1 + dx + W]
            e_k = ew[:, k, :].unsqueeze(1).broadcast_to([128, C, W])
            if k == 0:
                nc.vector.tensor_mul(acc[:, :, :], f_k, e_k)
            else:
                nc.vector.tensor_mul(diff[:, :, :], f_k, e_k)
                nc.vector.tensor_add(acc[:, :, :], acc[:, :, :],
                                     diff[:, :, :])
        nc.vector.tensor_sub(acc[:, :, :], acc[:, :, :], x_sb[:, :, :])

        # ---- store (transposing: [h, c, w] -> DRAM [c, h, w]) ----
        nc.sync.dma_start(out=o_hcw, in_=acc[:, :, :])

    return _split_sync_waits(nc)


class _SpmdRunner:
    """Executes the Bass graph SPMD on the 8 cores via PJRT/shard_map.

    Unlike bass2jax.run_bass_via_pjrt, inputs are device_put per-device and
    assembled with make_array_from_single_device_arrays, so JAX never
    compiles a dynamic-slice resharding program (neuronx-cc crashes building
    one for 32MB arrays).  The jitted executable is cached across calls.
    """

    def __init__(self, nc, n_cores):
        import jax
        from jax.experimental.shard_map import shard_map
        from jax.sharding import Mesh, NamedSharding, PartitionSpec

        from concourse import bass2jax as b2j

        b2j.install_neuronx_cc_hook()
        self.nc = nc
        self.n_cores = n_cores
        partition_name = (
            nc.partition_id_tensor.name if nc.partition_id_tensor else None
        )

        in_names, out_names, out_avals = [], [], []
        for alloc in nc.m.functions[0].allocations:
            if not isinstance(alloc, mybir.MemoryLocationSet):
                continue
            name = alloc.memorylocations[0].name
            if alloc.kind == "ExternalInput":
                if name != partition_name:
                    in_names.append(name)
            elif alloc.kind == "ExternalOutput":
                out_names.append(name)
                out_avals.append(
                    jax.core.ShapedArray(
                        tuple(alloc.tensor_shape), mybir.dt.np(alloc.dtype)
                    )
                )
        self.in_names, self.out_names = in_names, out_names
        self.out_avals = out_avals
        n_params, n_outs = len(in_names), len(out_names)
        all_in_names = in_names + out_names + (
            [partition_name] if partition_name else []
        )

        def _body(*args):
            operands = list(args)
            if partition_name is not None:
                operands.append(b2j.partition_id_tensor())
            outs = b2j._bass_exec_p.bind(
                *operands,
                out_avals=tuple(out_avals),
                in_names=tuple(all_in_names),
                out_names=tuple(out_names),
                lowering_input_output_aliases=(),
                sim_require_finite=True,
                sim_require_nnan=True,
                nc=nc,
            )
            return tuple(outs)

        self.devices = jax.devices()[:n_cores]
        assert len(self.devices) == n_cores
        mesh = Mesh(np.asarray(self.devices), ("core",))
        self.sharding = NamedSharding(mesh, PartitionSpec("core"))
        self.sharded = jax.jit(
            shard_map(
                _body, mesh=mesh,
                in_specs=(PartitionSpec("core"),) * (n_params + n_outs),
                out_specs=(PartitionSpec("core"),) * n_outs,
                check_rep=False,
            ),
            donate_argnums=tuple(range(n_params, n_params + n_outs)),
            keep_unused=True,
        )

    def _make_global(self, shards_np):
        import jax

        shards = [
            jax.device_put(s, self.devices[c])
            for c, s in enumerate(shards_np)
        ]
        gshape = (self.n_cores * shards_np[0].shape[0],) + tuple(
            shards_np[0].shape[1:]
        )
        return jax.make_array_from_single_device_arrays(
            gshape, self.sharding, shards
        )

    def __call__(self, in_maps):
        gin = [
            self._make_global(
                [np.asarray(in_maps[c][name]) for c in range(self.n_cores)]
            )
            for name in self.in_names
        ]
        gzero = [
            self._make_global(
                [np.zeros(a.shape, a.dtype) for _ in range(self.n_cores)]
            )
            for a in self.out_avals
        ]
        out_arrs = self.sharded(*gin, *gzero)
        results = [dict() for _ in range(self.n_cores)]
        for i, name in enumerate(self.out_names):
            for sh in out_arrs[i].addressable_shards:
                results[self.devices.index(sh.device)][name] = np.asarray(
                    sh.data
                )
        return results


def _get_runner():
    if "runner" not in _cache:
        _cache["runner"] = _SpmdRunner(_build_kernel(), N_CORES)
    return _cache["runner"]


def kernel(fe_lv, fused_features):
    fe_lv = np.asarray(fe_lv, dtype=np.float32)
    fused_features = np.asarray(fused_features, dtype=np.float32)

    runner = _get_runner()
    in_maps = [
        {
            "fe_lv": np.ascontiguousarray(fe_lv[i]),
            "fused_features": np.ascontiguousarray(fused_features[i]),
        }
        for i in range(N_CORES)
    ]
    results = runner(in_maps)
    out = np.stack([results[i]["out"] for i in range(N_CORES)], axis=0)
    return out


def bench(fe_lv, fused_features, trace_dir=None):
    """Run once (compiling/warming), then re-run under an NTFF profile
    capture and return (out, exec_time_ns, trace_info)."""
    import ctypes
    import glob as _glob
    import tempfile

    out = kernel(fe_lv, fused_features)
    runner = _cache["runner"]

    neff_dir = trace_dir or tempfile.mkdtemp(prefix="ntff_prof_")
    lib = ctypes.CDLL("/opt/axon/libaxon_pjrt.so")
    if not hasattr(lib, "axon_start_nrt_profile"):
        return out, None, "no axon_start_nrt_profile symbol"
    lib.axon_start_nrt_profile.argtypes = [
        ctypes.POINTER(ctypes.c_int64), ctypes.c_size_t,
    ]
    lib.axon_start_nrt_profile.restype = ctypes.c_int64
    lib.axon_stop_nrt_profile.argtypes = [ctypes.c_char_p]
    lib.axon_stop_nrt_profile.restype = ctypes.c_int64

    in_maps = [
        {
            "fe_lv": np.ascontiguousarray(np.asarray(fe_lv[i], np.float32)),
            "fused_features": np.ascontiguousarray(
                np.asarray(fused_features[i], np.float32)),
        }
        for i in range(N_CORES)
    ]
    rc = lib.axon_start_nrt_profile(None, 0)
    if rc != 0:
        return out, None, f"axon_start_nrt_profile rc={rc}"
    runner(in_maps)
    n = lib.axon_stop_nrt_profile(neff_dir.encode())
    if n <= 0:
        return out, None, f"axon_stop_nrt_profile rc={n}"

    ntffs = _glob.glob(os.path.join(neff_dir, "*_body*.ntff"))
    if not ntffs:
        return out, None, f"no *_body*.ntff in {neff_dir}: " + str(
            sorted(os.listdir(neff_dir)))

    import gauge.profiler
    from concourse._compat import FishPath

    profile = gauge.profiler.Profile(
        profile_path=FishPath(neff_dir),
        kernel_dev_mode=True,
        profile_on_exit=False,
        bass_kernel=_cache["runner"].nc.m,
        offline_processing=True,
        fname="*_body*",
    )
    perfetto_results = profile.to_perfetto(model_index=(0,))
    if not perfetto_results:
        return out, None, f"no perfetto results ({neff_dir})"
    pr = perfetto_results[0]
    return out, pr.exec_time_ns, {"trace_path": pr.trace_path,
                                  "neff_dir": neff_dir}


# revision 23
# speedup vs baseline: 2.2141x; 2.2141x over previous
"""AdaptiveSpectralFeatureRefinement (Euclidean) — Trainium2 Bass kernel.

Reference op (per batch element b):
  patches = unfold3x3(fused_features)                 # [C, 9, H, W]
  dist_k  = || patches_k - fe_lv ||_2  (over C)       # [9, H, W]
  w       = softmax_k(-dist_k)
  out     = sum_k w_k * patches_k + fe_lv             # [C, H, W]

Sharding: data-parallel over batch B=8 across the 8 NeuronCores (the op is
fully local per batch element, no collectives needed).

Per-core layout: partitions = w (128 cols), free = (h, c) with c innermost.
  - DRAM loads are *natural* (partition = c, 64KB contiguous runs) and then
    transposed on-chip through the TensorEngine ([64,128] blocks -> [128,64])
    because a direct transposing DMA load needs 8192 x 512B descriptors which
    overflows the compiler's 16-bit DMA semaphore field.
  - dx (col) shifts -> three partition-shifted copies of f (SBUF->SBUF DMA)
  - dy (row) shifts -> free-dim offset slices into an h-padded [*, H+2, C] tile
"""

import sys

if "/opt/trn_rl_repo" not in sys.path:
    sys.path.insert(0, "/opt/trn_rl_repo")

import os
from contextlib import ExitStack

import numpy as np

import concourse.bass as bass
import concourse.tile as tile
from concourse import mybir
from concourse.bass_utils import run_bass_kernel_spmd
from concourse.masks import make_identity

B, C, H, W = 8, 64, 128, 128
N_CORES = 8
FP = mybir.dt.float32
BF = mybir.dt.bfloat16
ACT = mybir.ActivationFunctionType

_cache = {}


def _split_sync_waits(nc, max_waits=1):
    """This container's walrus codegen accepts at most one sync-wait command
    per instruction, but Tile emits up to ~3 on instructions with multiple
    cross-engine producers.  Legalize by hoisting the extra waits into NoOps
    on the same engine, inserted immediately before the instruction (engine
    streams execute in block order, so the waits still gate it)."""
    for f in nc.m.functions:
        for blk in f.blocks:
            new_insts = []
            changed = False
            for inst in blk.instructions:
                si = getattr(inst, "sync_info", None)
                if si is not None and si.on_wait and len(si.on_wait) > max_waits:
                    waits = list(si.on_wait)
                    for i, w in enumerate(waits[max_waits:]):
                        nop = mybir.InstNoOp(
                            name=f"{inst.name}_ws{i}",
                            engine=inst.engine,
                            sync_info=mybir.SyncInfo(on_wait=[w],
                                                     on_update=[]),
                            bass_nofuse=True,
                        )
                        new_insts.append(nop)
                    inst.sync_info = mybir.SyncInfo(
                        on_wait=waits[:max_waits],
                        on_update=list(si.on_update),
                    )
                    changed = True
                new_insts.append(inst)
            if changed:
                blk.instructions = new_insts
    return nc


def _build_kernel():
    nc = bass.Bass("TRN2", target_bir_lowering=False, debug=False,
                   num_devices=N_CORES)

    x_d = nc.dram_tensor("fe_lv", [C, H, W], FP, kind="ExternalInput").ap()
    f_d = nc.dram_tensor("fused_features", [C, H, W], FP,
                         kind="ExternalInput").ap()
    o_d = nc.dram_tensor("out", [C, H, W], FP, kind="ExternalOutput").ap()

    # DRAM APs reordered to [h, c, w] so partition dim = h.
    x_hcw = x_d.transpose([1, 0, 2])
    f_hcw = f_d.transpose([1, 0, 2])
    o_hcw = o_d.transpose([1, 0, 2])

    with tile.TileContext(nc) as tc, ExitStack() as ctx:
        main = ctx.enter_context(tc.tile_pool(name="main", bufs=1))
        tp = ctx.enter_context(tc.tile_pool(name="tp", bufs=2))
        psum = ctx.enter_context(tc.tile_pool(name="psum", bufs=1,
                                              space="PSUM"))

        # Persistent tiles; layout [h(part), c, w(+2 pad)], compute in bf16.
        xs_f32 = main.tile([128, C, W], FP, tag="bigf32")  # stage; reused: out
        fs_f32 = main.tile([128, C, W], FP)                # f stage
        x_bf = main.tile([128, C, W], BF)                  # -x in bf16
        f_m1 = main.tile([128, C, W + 2], BF)              # f rows h-1
        f_c0 = main.tile([128, C, W + 2], BF)              # f rows h
        f_p1 = main.tile([128, C, W + 2], BF)              # f rows h+1
        dist = main.tile([128, 9, W], FP)                  # dist^2 -> dist
        ew = main.tile([128, 9, W], FP)                    # exp weights (f32)
        ewb = main.tile([128, 9, W], BF)                   # exp weights (bf16)
        mmin = main.tile([128, W], FP)
        ssum = main.tile([128, W], FP)
        ident = main.tile([128, 128], BF)                  # PE accumulation id

        f_dy = {-1: f_m1, 0: f_c0, 1: f_p1}

        # ---- loads (parallel HWDGE queues) + bf16 casts ----
        nc.sync.dma_start(out=xs_f32[:, :, :], in_=x_hcw)
        nc.scalar.dma_start(out=fs_f32[:, :, :], in_=f_hcw)
        # cast + negate x (phase 1 computes f_k + (-x); residual: acc - (-x))
        nc.vector.tensor_scalar_mul(x_bf[:, :, :], xs_f32[:, :, :], -1.0)
        # f tiles: full-tile memset (zero pads + boundary rows), cast into
        # interior, then partition-shifted SBUF->SBUF copies for dy = +-1.
        nc.vector.memset(f_c0[:, :, :], 0.0)
        nc.vector.memset(f_m1[:, :, :], 0.0)
        nc.vector.memset(f_p1[:, :, :], 0.0)
        nc.vector.tensor_copy(f_c0[:, :, 1:W + 1], fs_f32[:, :, :])
        nc.gpsimd.dma_start(out=f_m1[1:128, :, :], in_=f_c0[0:127, :, :])
        nc.gpsimd.dma_start(out=f_p1[0:127, :, :], in_=f_c0[1:128, :, :])

        make_identity(nc, ident[:, :])

        # ---- phase 1: dist^2 for the 9 neighbors ----
        # per k: DVE sub (bf16 2x) -> ACT square (overlaps next k's sub via
        # double-buffered t tiles) -> DVE pairwise tree reduction over c.
        for k in range(9):
            dy, dx = k // 3 - 1, k % 3 - 1
            f_k = f_dy[dy][:, :, 1 + dx:1 + dx + W]
            t = tp.tile([128, C, W], BF, tag="t")
            nc.vector.tensor_add(t[:, :, :], f_k, x_bf[:, :, :])
            nc.scalar.activation(t[:, :, :], t[:, :, :], ACT.Square)
            c2 = C // 2
            while c2 >= 2:
                nc.vector.tensor_add(t[:, 0:c2, :], t[:, 0:c2, :],
                                     t[:, c2:2 * c2, :])
                c2 //= 2
            nc.vector.tensor_add(dist[:, k, :], t[:, 0, :], t[:, 1, :])

        # ---- phase 2: softmax over 9 neighbors of -sqrt(dist2) (f32) ----
        nc.vector.tensor_reduce(
            out=mmin[:, :], in_=dist[:, :, :].transpose([0, 2, 1]),
            axis=mybir.AxisListType.X, op=mybir.AluOpType.min,
        )
        nc.scalar.activation(dist[:, :, :], dist[:, :, :], ACT.Sqrt)
        nc.scalar.activation(mmin[:, :], mmin[:, :], ACT.Sqrt)
        # e = exp(dmin - d) (<= 1, no overflow)
        nc.vector.tensor_sub(
            ew[:, :, :],
            mmin[:, :].unsqueeze(1).broadcast_to([128, 9, W]),
            dist[:, :, :],
        )
        nc.scalar.activation(ew[:, :, :], ew[:, :, :], ACT.Exp)
        nc.vector.tensor_reduce(
            out=ssum[:, :], in_=ew[:, :, :].transpose([0, 2, 1]),
            axis=mybir.AxisListType.X, op=mybir.AluOpType.add,
        )
        nc.vector.reciprocal(ssum[:, :], ssum[:, :])
        nc.vector.tensor_mul(
            ew[:, :, :], ew[:, :, :],
            ssum[:, :].unsqueeze(1).broadcast_to([128, 9, W]),
        )
        nc.vector.tensor_copy(ewb[:, :, :], ew[:, :, :])

        # ---- phase 3: weighted sum via DVE mults + PE accumulation ----
        # Processed in two c-groups so the first group's store overlaps the
        # second group's compute.  PSUM accumulator: identity matmul
        # out[m, n] += sum_p I[p, m] * t[p, n].
        out_f32 = xs_f32  # staging tile reused as the f32 output
        CG = C // 2
        for g in range(2):
            c0 = g * CG
            pacc = psum.tile([128, CG * W], FP, tag="pacc")
            for k in range(9):
                dy, dx = k // 3 - 1, k % 3 - 1
                f_k = f_dy[dy][:, c0:c0 + CG, 1 + dx:1 + dx + W]
                e_k = ewb[:, k, :].unsqueeze(1).broadcast_to([128, CG, W])
                t2 = tp.tile([128, CG, W], BF, tag="t2")
                nc.vector.tensor_mul(t2[:, :, :], f_k, e_k)
                t2f = t2[:, :, :].rearrange("p c w -> p (c w)")
                for ch in range(CG * W // 512):
                    nc.tensor.matmul(
                        pacc[:, ch * 512:(ch + 1) * 512],
                        ident[:, :],
                        t2f[:, ch * 512:(ch + 1) * 512],
                        start=(k == 0), stop=(k == 8),
                    )
            # residual: out = pacc - (-x)
            nc.vector.tensor_sub(
                out_f32[:, c0:c0 + CG, :],
                pacc[:, :].rearrange("p (c w) -> p c w", c=CG),
                x_bf[:, c0:c0 + CG, :],
            )
            # store this c-group (overlaps next group's compute)
            h_half = CG // 2
            nc.sync.dma_start(out=o_hcw[:, c0:c0 + h_half, :],
                              in_=out_f32[:, c0:c0 + h_half, :])
            nc.scalar.dma_start(
                out=o_hcw[:, c0 + h_half:c0 + CG, :],
                in_=out_f32[:, c0 + h_half:c0 + CG, :])

    return _split_sync_waits(nc)


class _SpmdRunner:
    """Executes the Bass graph SPMD on the 8 cores via PJRT/shard_map.

    Unlike bass2jax.run_bass_via_pjrt, inputs are device_put per-device and
    assembled with make_array_from_single_device_arrays, so JAX never
    compiles a dynamic-slice resharding program (neuronx-cc crashes building
    one for 32MB arrays).  The jitted executable is cached across calls.
    """

    def __init__(self, nc, n_cores):
        import jax
        from jax.experimental.shard_map import shard_map
        from jax.sharding import Mesh, NamedSharding, PartitionSpec

        from concourse import bass2jax as b2j

        b2j.install_neuronx_cc_hook()
        self.nc = nc
        self.n_cores = n_cores
        partition_name = (
            nc.partition_id_tensor.name if nc.partition_id_tensor else None
        )

        in_names, out_names, out_avals = [], [], []
        for alloc in nc.m.functions[0].allocations:
            if not isinstance(alloc, mybir.MemoryLocationSet):
                continue
            name = alloc.memorylocations[0].name
            if alloc.kind == "ExternalInput":
                if name != partition_name:
                    in_names.append(name)
            elif alloc.kind == "ExternalOutput":
                out_names.append(name)
                out_avals.append(
                    jax.core.ShapedArray(
                        tuple(alloc.tensor_shape), mybir.dt.np(alloc.dtype)
                    )
                )
        self.in_names, self.out_names = in_names, out_names
        self.out_avals = out_avals
        n_params, n_outs = len(in_names), len(out_names)
        all_in_names = in_names + out_names + (
            [partition_name] if partition_name else []
        )

        def _body(*args):
            operands = list(args)
            if partition_name is not None:
                operands.append(b2j.partition_id_tensor())
            outs = b2j._bass_exec_p.bind(
                *operands,
                out_avals=tuple(out_avals),
                in_names=tuple(all_in_names),
                out_names=tuple(out_names),
                lowering_input_output_aliases=(),
                sim_require_finite=True,
                sim_require_nnan=True,
                nc=nc,
            )
            return tuple(outs)

        self.devices = jax.devices()[:n_cores]
        assert len(self.devices) == n_cores
        mesh = Mesh(np.asarray(self.devices), ("core",))
        self.sharding = NamedSharding(mesh, PartitionSpec("core"))
        self.sharded = jax.jit(
            shard_map(
                _body, mesh=mesh,
                in_specs=(PartitionSpec("core"),) * (n_params + n_outs),
                out_specs=(PartitionSpec("core"),) * n_outs,
                check_rep=False,
            ),
            donate_argnums=tuple(range(n_params, n_params + n_outs)),
            keep_unused=True,
        )

    def _make_global(self, shards_np):
        import jax

        shards = [
            jax.device_put(s, self.devices[c])
            for c, s in enumerate(shards_np)
        ]
        gshape = (self.n_cores * shards_np[0].shape[0],) + tuple(
            shards_np[0].shape[1:]
        )
        return jax.make_array_from_single_device_arrays(
            gshape, self.sharding, shards
        )

    def __call__(self, in_maps):
        gin = [
            self._make_global(
                [np.asarray(in_maps[c][name]) for c in range(self.n_cores)]
            )
            for name in self.in_names
        ]
        gzero = [
            self._make_global(
                [np.zeros(a.shape, a.dtype) for _ in range(self.n_cores)]
            )
            for a in self.out_avals
        ]
        out_arrs = self.sharded(*gin, *gzero)
        results = [dict() for _ in range(self.n_cores)]
        for i, name in enumerate(self.out_names):
            for sh in out_arrs[i].addressable_shards:
                results[self.devices.index(sh.device)][name] = np.asarray(
                    sh.data
                )
        return results


def _get_runner():
    if "runner" not in _cache:
        _cache["runner"] = _SpmdRunner(_build_kernel(), N_CORES)
    return _cache["runner"]


def kernel(fe_lv, fused_features):
    fe_lv = np.asarray(fe_lv, dtype=np.float32)
    fused_features = np.asarray(fused_features, dtype=np.float32)

    runner = _get_runner()
    in_maps = [
        {
            "fe_lv": np.ascontiguousarray(fe_lv[i]),
            "fused_features": np.ascontiguousarray(fused_features[i]),
        }
        for i in range(N_CORES)
    ]
    results = runner(in_maps)
    out = np.stack([results[i]["out"] for i in range(N_CORES)], axis=0)
    return out


def bench(fe_lv, fused_features, trace_dir=None):
    """Run once (compiling/warming), then re-run under an NTFF profile
    capture and return (out, exec_time_ns, trace_info)."""
    import ctypes
    import glob as _glob
    import tempfile

    out = kernel(fe_lv, fused_features)
    runner = _cache["runner"]

    neff_dir = trace_dir or tempfile.mkdtemp(prefix="ntff_prof_")
    lib = ctypes.CDLL("/opt/axon/libaxon_pjrt.so")
    if not hasattr(lib, "axon_start_nrt_profile"):
        return out, None, "no axon_start_nrt_profile symbol"
    lib.axon_start_nrt_profile.argtypes = [
        ctypes.POINTER(ctypes.c_int64), ctypes.c_size_t,
    ]
    lib.axon_start_nrt_profile.restype = ctypes.c_int64
    lib.axon_stop_nrt_profile.argtypes = [ctypes.c_char_p]
    lib.axon_stop_nrt_profile.restype = ctypes.c_int64

    in_maps = [
        {
            "fe_lv": np.ascontiguousarray(np.asarray(fe_lv[i], np.float32)),
            "fused_features": np.ascontiguousarray(
                np.asarray(fused_features[i], np.float32)),
        }
        for i in range(N_CORES)
    ]
    rc = lib.axon_start_nrt_profile(None, 0)
    if rc != 0:
        return out, None, f"axon_start_nrt_profile rc={rc}"
    runner(in_maps)
    n = lib.axon_stop_nrt_profile(neff_dir.encode())
    if n <= 0:
        return out, None, f"axon_stop_nrt_profile rc={n}"

    ntffs = _glob.glob(os.path.join(neff_dir, "*_body*.ntff"))
    if not ntffs:
        return out, None, f"no *_body*.ntff in {neff_dir}: " + str(
            sorted(os.listdir(neff_dir)))

    import gauge.profiler
    from concourse._compat import FishPath

    profile = gauge.profiler.Profile(
        profile_path=FishPath(neff_dir),
        kernel_dev_mode=True,
        profile_on_exit=False,
        bass_kernel=_cache["runner"].nc.m,
        offline_processing=True,
        fname="*_body*",
    )
    perfetto_results = profile.to_perfetto(model_index=(0,))
    if not perfetto_results:
        return out, None, f"no perfetto results ({neff_dir})"
    pr = perfetto_results[0]
    return out, pr.exec_time_ns, {"trace_path": pr.trace_path,
                                  "neff_dir": neff_dir}


# revision 26
# speedup vs baseline: 2.6043x; 1.1763x over previous
"""AdaptiveSpectralFeatureRefinement (Euclidean) — Trainium2 Bass kernel.

Reference op (per batch element b):
  patches = unfold3x3(fused_features)                 # [C, 9, H, W]
  dist_k  = || patches_k - fe_lv ||_2  (over C)       # [9, H, W]
  w       = softmax_k(-dist_k)
  out     = sum_k w_k * patches_k + fe_lv             # [C, H, W]

Sharding: data-parallel over batch B=8 across the 8 NeuronCores (the op is
fully local per batch element, no collectives needed).

Per-core layout: partitions = w (128 cols), free = (h, c) with c innermost.
  - DRAM loads are *natural* (partition = c, 64KB contiguous runs) and then
    transposed on-chip through the TensorEngine ([64,128] blocks -> [128,64])
    because a direct transposing DMA load needs 8192 x 512B descriptors which
    overflows the compiler's 16-bit DMA semaphore field.
  - dx (col) shifts -> three partition-shifted copies of f (SBUF->SBUF DMA)
  - dy (row) shifts -> free-dim offset slices into an h-padded [*, H+2, C] tile
"""

import sys

if "/opt/trn_rl_repo" not in sys.path:
    sys.path.insert(0, "/opt/trn_rl_repo")

import os
from contextlib import ExitStack

import numpy as np

import concourse.bass as bass
import concourse.tile as tile
from concourse import mybir
from concourse.bass_utils import run_bass_kernel_spmd
from concourse.masks import make_identity

B, C, H, W = 8, 64, 128, 128
N_CORES = 8
FP = mybir.dt.float32
BF = mybir.dt.bfloat16
ACT = mybir.ActivationFunctionType

_cache = {}


def _split_sync_waits(nc, max_waits=1):
    """This container's walrus codegen accepts at most one sync-wait command
    per instruction, but Tile emits up to ~3 on instructions with multiple
    cross-engine producers.  Legalize by hoisting the extra waits into NoOps
    on the same engine, inserted immediately before the instruction (engine
    streams execute in block order, so the waits still gate it)."""
    for f in nc.m.functions:
        for blk in f.blocks:
            new_insts = []
            changed = False
            for inst in blk.instructions:
                si = getattr(inst, "sync_info", None)
                if si is not None and si.on_wait and len(si.on_wait) > max_waits:
                    waits = list(si.on_wait)
                    for i, w in enumerate(waits[max_waits:]):
                        nop = mybir.InstNoOp(
                            name=f"{inst.name}_ws{i}",
                            engine=inst.engine,
                            sync_info=mybir.SyncInfo(on_wait=[w],
                                                     on_update=[]),
                            bass_nofuse=True,
                        )
                        new_insts.append(nop)
                    inst.sync_info = mybir.SyncInfo(
                        on_wait=waits[:max_waits],
                        on_update=list(si.on_update),
                    )
                    changed = True
                new_insts.append(inst)
            if changed:
                blk.instructions = new_insts
    return nc


def _build_kernel():
    nc = bass.Bass("TRN2", target_bir_lowering=False, debug=False,
                   num_devices=N_CORES)

    x_d = nc.dram_tensor("fe_lv", [C, H, W], FP, kind="ExternalInput").ap()
    f_d = nc.dram_tensor("fused_features", [C, H, W], FP,
                         kind="ExternalInput").ap()
    o_d = nc.dram_tensor("out", [C, H, W], FP, kind="ExternalOutput").ap()

    # DRAM APs reordered to [h, c, w] so partition dim = h.
    x_hcw = x_d.transpose([1, 0, 2])
    f_hcw = f_d.transpose([1, 0, 2])
    o_hcw = o_d.transpose([1, 0, 2])

    with tile.TileContext(nc) as tc, ExitStack() as ctx:
        main = ctx.enter_context(tc.tile_pool(name="main", bufs=1))
        tp = ctx.enter_context(tc.tile_pool(name="tp", bufs=3))
        tp2 = ctx.enter_context(tc.tile_pool(name="tp2", bufs=2))
        psum = ctx.enter_context(tc.tile_pool(name="psum", bufs=1,
                                              space="PSUM"))

        # Persistent tiles; layout [h(part), c, w(+2 pad)], compute in bf16.
        xs_f32 = main.tile([128, C, W], FP, tag="bigf32")  # stage; reused: out
        fs_f32 = main.tile([128, C, W], FP)                # f stage
        x_bf = main.tile([128, C, W], BF)                  # -x in bf16
        f_m1 = main.tile([128, C, W + 2], BF)              # f rows h-1
        f_c0 = main.tile([128, C, W + 2], BF)              # f rows h
        f_p1 = main.tile([128, C, W + 2], BF)              # f rows h+1
        dist = main.tile([128, 9, W], FP)                  # dist^2 -> dist
        ew = main.tile([128, 9, W], FP)                    # exp weights (f32)
        ewb = main.tile([128, 9, W], BF)                   # exp weights (bf16)
        mmin = main.tile([128, W], FP)
        ssum = main.tile([128, W], FP)
        ident = main.tile([128, 128], BF)                  # PE accumulation id

        f_dy = {-1: f_m1, 0: f_c0, 1: f_p1}

        # ---- loads (parallel HWDGE queues) + bf16 casts ----
        nc.sync.dma_start(out=xs_f32[:, :, :], in_=x_hcw)
        nc.scalar.dma_start(out=fs_f32[:, :, :], in_=f_hcw)
        # cast + negate x (phase 1 computes f_k + (-x); residual: acc - (-x))
        nc.vector.tensor_scalar_mul(x_bf[:, :, :], xs_f32[:, :, :], -1.0)
        # f tiles: full-tile memset (zero pads + boundary rows), cast into
        # interior, then partition-shifted SBUF->SBUF copies for dy = +-1.
        nc.vector.memset(f_c0[:, :, :], 0.0)
        nc.vector.memset(f_m1[:, :, :], 0.0)
        nc.vector.memset(f_p1[:, :, :], 0.0)
        nc.vector.tensor_copy(f_c0[:, :, 1:W + 1], fs_f32[:, :, :])
        nc.gpsimd.dma_start(out=f_m1[1:128, :, :], in_=f_c0[0:127, :, :])
        nc.gpsimd.dma_start(out=f_p1[0:127, :, :], in_=f_c0[1:128, :, :])

        make_identity(nc, ident[:, :])

        # ---- phase 1: dist^2 for the 9 neighbors ----
        # per k: DVE sub (bf16 2x) -> ACT square (overlaps next k's sub via
        # double-buffered t tiles) -> DVE pairwise tree reduction over c.
        for k in range(9):
            dy, dx = k // 3 - 1, k % 3 - 1
            f_k = f_dy[dy][:, :, 1 + dx:1 + dx + W]
            t = tp.tile([128, C, W], BF, tag="t")
            nc.vector.tensor_add(t[:, :, :], f_k, x_bf[:, :, :])
            nc.scalar.activation(t[:, :, :], t[:, :, :], ACT.Square)
            c2 = C // 2
            while c2 >= 2:
                nc.vector.tensor_add(t[:, 0:c2, :], t[:, 0:c2, :],
                                     t[:, c2:2 * c2, :])
                c2 //= 2
            nc.vector.tensor_add(dist[:, k, :], t[:, 0, :], t[:, 1, :])

        # ---- phase 2: softmax over 9 neighbors of -sqrt(dist2) (f32) ----
        nc.vector.tensor_reduce(
            out=mmin[:, :], in_=dist[:, :, :].transpose([0, 2, 1]),
            axis=mybir.AxisListType.X, op=mybir.AluOpType.min,
        )
        nc.scalar.activation(dist[:, :, :], dist[:, :, :], ACT.Sqrt)
        nc.scalar.activation(mmin[:, :], mmin[:, :], ACT.Sqrt)
        # e = exp(dmin - d) (<= 1, no overflow)
        nc.vector.tensor_sub(
            ew[:, :, :],
            mmin[:, :].unsqueeze(1).broadcast_to([128, 9, W]),
            dist[:, :, :],
        )
        nc.scalar.activation(ew[:, :, :], ew[:, :, :], ACT.Exp)
        nc.vector.tensor_reduce(
            out=ssum[:, :], in_=ew[:, :, :].transpose([0, 2, 1]),
            axis=mybir.AxisListType.X, op=mybir.AluOpType.add,
        )
        nc.vector.reciprocal(ssum[:, :], ssum[:, :])
        nc.vector.tensor_mul(
            ew[:, :, :], ew[:, :, :],
            ssum[:, :].unsqueeze(1).broadcast_to([128, 9, W]),
        )
        nc.vector.tensor_copy(ewb[:, :, :], ew[:, :, :])

        # ---- phase 3: weighted sum via DVE mults + PE accumulation ----
        # Processed in two c-groups so the first group's store overlaps the
        # second group's compute.  PSUM accumulator: identity matmul
        # out[m, n] += sum_p I[p, m] * t[p, n].
        out_f32 = xs_f32  # staging tile reused as the f32 output
        CG = C // 2
        for g in range(2):
            c0 = g * CG
            pacc = psum.tile([128, CG * W], FP, tag="pacc")
            for k in range(9):
                dy, dx = k // 3 - 1, k % 3 - 1
                f_k = f_dy[dy][:, c0:c0 + CG, 1 + dx:1 + dx + W]
                e_k = ewb[:, k, :].unsqueeze(1).broadcast_to([128, CG, W])
                t2 = tp2.tile([128, CG, W], BF, tag="t2")
                nc.vector.tensor_mul(t2[:, :, :], f_k, e_k)
                t2f = t2[:, :, :].rearrange("p c w -> p (c w)")
                for ch in range(CG * W // 512):
                    nc.tensor.matmul(
                        pacc[:, ch * 512:(ch + 1) * 512],
                        ident[:, :],
                        t2f[:, ch * 512:(ch + 1) * 512],
                        start=(k == 0), stop=(k == 8),
                    )
            # residual: out = pacc - (-x)
            nc.vector.tensor_sub(
                out_f32[:, c0:c0 + CG, :],
                pacc[:, :].rearrange("p (c w) -> p c w", c=CG),
                x_bf[:, c0:c0 + CG, :],
            )
            # store this c-group (overlaps next group's compute)
            h_half = CG // 2
            nc.sync.dma_start(out=o_hcw[:, c0:c0 + h_half, :],
                              in_=out_f32[:, c0:c0 + h_half, :])
            nc.scalar.dma_start(
                out=o_hcw[:, c0 + h_half:c0 + CG, :],
                in_=out_f32[:, c0 + h_half:c0 + CG, :])

    return _split_sync_waits(nc)


class _SpmdRunner:
    """Executes the Bass graph SPMD on the 8 cores via PJRT/shard_map.

    Unlike bass2jax.run_bass_via_pjrt, inputs are device_put per-device and
    assembled with make_array_from_single_device_arrays, so JAX never
    compiles a dynamic-slice resharding program (neuronx-cc crashes building
    one for 32MB arrays).  The jitted executable is cached across calls.
    """

    def __init__(self, nc, n_cores):
        import jax
        from jax.experimental.shard_map import shard_map
        from jax.sharding import Mesh, NamedSharding, PartitionSpec

        from concourse import bass2jax as b2j

        b2j.install_neuronx_cc_hook()
        self.nc = nc
        self.n_cores = n_cores
        partition_name = (
            nc.partition_id_tensor.name if nc.partition_id_tensor else None
        )

        in_names, out_names, out_avals = [], [], []
        for alloc in nc.m.functions[0].allocations:
            if not isinstance(alloc, mybir.MemoryLocationSet):
                continue
            name = alloc.memorylocations[0].name
            if alloc.kind == "ExternalInput":
                if name != partition_name:
                    in_names.append(name)
            elif alloc.kind == "ExternalOutput":
                out_names.append(name)
                out_avals.append(
                    jax.core.ShapedArray(
                        tuple(alloc.tensor_shape), mybir.dt.np(alloc.dtype)
                    )
                )
        self.in_names, self.out_names = in_names, out_names
        self.out_avals = out_avals
        n_params, n_outs = len(in_names), len(out_names)
        all_in_names = in_names + out_names + (
            [partition_name] if partition_name else []
        )

        def _body(*args):
            operands = list(args)
            if partition_name is not None:
                operands.append(b2j.partition_id_tensor())
            outs = b2j._bass_exec_p.bind(
                *operands,
                out_avals=tuple(out_avals),
                in_names=tuple(all_in_names),
                out_names=tuple(out_names),
                lowering_input_output_aliases=(),
                sim_require_finite=True,
                sim_require_nnan=True,
                nc=nc,
            )
            return tuple(outs)

        self.devices = jax.devices()[:n_cores]
        assert len(self.devices) == n_cores
        mesh = Mesh(np.asarray(self.devices), ("core",))
        self.sharding = NamedSharding(mesh, PartitionSpec("core"))
        self.sharded = jax.jit(
            shard_map(
                _body, mesh=mesh,
                in_specs=(PartitionSpec("core"),) * (n_params + n_outs),
                out_specs=(PartitionSpec("core"),) * n_outs,
                check_rep=False,
            ),
            donate_argnums=tuple(range(n_params, n_params + n_outs)),
            keep_unused=True,
        )

    def _make_global(self, shards_np):
        import jax

        shards = [
            jax.device_put(s, self.devices[c])
            for c, s in enumerate(shards_np)
        ]
        gshape = (self.n_cores * shards_np[0].shape[0],) + tuple(
            shards_np[0].shape[1:]
        )
        return jax.make_array_from_single_device_arrays(
            gshape, self.sharding, shards
        )

    def __call__(self, in_maps):
        gin = [
            self._make_global(
                [np.asarray(in_maps[c][name]) for c in range(self.n_cores)]
            )
            for name in self.in_names
        ]
        gzero = [
            self._make_global(
                [np.zeros(a.shape, a.dtype) for _ in range(self.n_cores)]
            )
            for a in self.out_avals
        ]
        out_arrs = self.sharded(*gin, *gzero)
        results = [dict() for _ in range(self.n_cores)]
        for i, name in enumerate(self.out_names):
            for sh in out_arrs[i].addressable_shards:
                results[self.devices.index(sh.device)][name] = np.asarray(
                    sh.data
                )
        return results


def _get_runner():
    if "runner" not in _cache:
        _cache["runner"] = _SpmdRunner(_build_kernel(), N_CORES)
    return _cache["runner"]


def kernel(fe_lv, fused_features):
    fe_lv = np.asarray(fe_lv, dtype=np.float32)
    fused_features = np.asarray(fused_features, dtype=np.float32)

    runner = _get_runner()
    in_maps = [
        {
            "fe_lv": np.ascontiguousarray(fe_lv[i]),
            "fused_features": np.ascontiguousarray(fused_features[i]),
        }
        for i in range(N_CORES)
    ]
    results = runner(in_maps)
    out = np.stack([results[i]["out"] for i in range(N_CORES)], axis=0)
    return out


def bench(fe_lv, fused_features, trace_dir=None):
    """Run once (compiling/warming), then re-run under an NTFF profile
    capture and return (out, exec_time_ns, trace_info)."""
    import ctypes
    import glob as _glob
    import tempfile

    out = kernel(fe_lv, fused_features)
    runner = _cache["runner"]

    neff_dir = trace_dir or tempfile.mkdtemp(prefix="ntff_prof_")
    lib = ctypes.CDLL("/opt/axon/libaxon_pjrt.so")
    if not hasattr(lib, "axon_start_nrt_profile"):
        return out, None, "no axon_start_nrt_profile symbol"
    lib.axon_start_nrt_profile.argtypes = [
        ctypes.POINTER(ctypes.c_int64), ctypes.c_size_t,
    ]
    lib.axon_start_nrt_profile.restype = ctypes.c_int64
    lib.axon_stop_nrt_profile.argtypes = [ctypes.c_char_p]
    lib.axon_stop_nrt_profile.restype = ctypes.c_int64

    in_maps = [
        {
            "fe_lv": np.ascontiguousarray(np.asarray(fe_lv[i], np.float32)),
            "fused_features": np.ascontiguousarray(
                np.asarray(fused_features[i], np.float32)),
        }
        for i in range(N_CORES)
    ]
    rc = lib.axon_start_nrt_profile(None, 0)
    if rc != 0:
        return out, None, f"axon_start_nrt_profile rc={rc}"
    runner(in_maps)
    n = lib.axon_stop_nrt_profile(neff_dir.encode())
    if n <= 0:
        return out, None, f"axon_stop_nrt_profile rc={n}"

    ntffs = _glob.glob(os.path.join(neff_dir, "*_body*.ntff"))
    if not ntffs:
        return out, None, f"no *_body*.ntff in {neff_dir}: " + str(
            sorted(os.listdir(neff_dir)))

    import gauge.profiler
    from concourse._compat import FishPath

    profile = gauge.profiler.Profile(
        profile_path=FishPath(neff_dir),
        kernel_dev_mode=True,
        profile_on_exit=False,
        bass_kernel=_cache["runner"].nc.m,
        offline_processing=True,
        fname="*_body*",
    )
    perfetto_results = profile.to_perfetto(model_index=(0,))
    if not perfetto_results:
        return out, None, f"no perfetto results ({neff_dir})"
    pr = perfetto_results[0]
    return out, pr.exec_time_ns, {"trace_path": pr.trace_path,
                                  "neff_dir": neff_dir}
